# revision 19
# baseline (speedup 1.0000x reference)
"""MHA layer (QKV proj + masked softmax attention + out proj + residual + LayerNorm)
on 8 NeuronCores. Sharding: batch(4) x query-half(2). No collectives: each core
computes K/V for its full batch, Q only for its half of T.

Self-contained: hardcodes shapes from the problem spec.
"""

import numpy as np

import concourse.bass as bass
import concourse.bacc as bacc
import concourse.tile as tile
import concourse.mybir as mybir
from concourse.bass_utils import run_bass_kernel_spmd

B, T, C, H, D = 4, 2048, 1024, 16, 64
TQ = T // 2          # query rows per core
N_CORES = 8
P = 128
NJ = C // P          # 8 c-chunks
NTK = T // P         # 16 key chunks
LN_EPS = 1e-5
VSLOT = 66           # V_aug per-head slot: 64 V cols + 1 ones + 1 pad

f32 = mybir.dt.float32
bf16 = mybir.dt.bfloat16
AX = mybir.AxisListType
ALU = mybir.AluOpType
ACTF = mybir.ActivationFunctionType


def build(affine: bool):
    import os as _os0
    phase_lim = int(_os0.environ.get("K_PHASE", "4"))
    n_reps = int(_os0.environ.get("K_REPS", "1"))
    nc = bacc.Bacc("TRN2", target_bir_lowering=False, debug=False,
                   num_devices=N_CORES)

    xbf = nc.dram_tensor("xbf", [T, C], bf16, kind="ExternalInput")
    w4 = nc.dram_tensor("w4", [4 * C, C], bf16, kind="ExternalInput")
    # fx0: xres rows; fxt rows: 0 bq; 1 bk; 2 bv; 3 bp; 4 lng; 5 lnb; 6 mask
    fx0 = nc.dram_tensor("fx0", [TQ, C], f32, kind="ExternalInput")
    fxt = nc.dram_tensor("fxt", [7, C], f32, kind="ExternalInput")
    wq = w4[0 * C:1 * C, :]
    wk = w4[1 * C:2 * C, :]
    wv = w4[2 * C:3 * C, :]
    wp = w4[3 * C:4 * C, :]
    xres = fx0[0:TQ, :]
    outd = nc.dram_tensor("out", [TQ, C], f32, kind="ExternalOutput")

    with tile.TileContext(nc) as tc:
        with (
            tc.tile_pool(name="pers", bufs=1) as pers,
            tc.tile_pool(name="big", bufs=1) as bigp,
            tc.tile_pool(name="wbig", bufs=1) as wbigp,
            tc.tile_pool(name="wsl", bufs=2) as wslp,
            tc.tile_pool(name="ev", bufs=2) as evp,
            tc.tile_pool(name="sm", bufs=2) as smp,
            tc.tile_pool(name="psum", bufs=1, space=bass.MemorySpace.PSUM) as psp,
        ):
            mrow_f = evp.tile([1, TQ], f32, tag="hres", bufs=2, name="mrow_f")
            nc.sync.dma_start(mrow_f[:], fxt[6:7, :])
            mrow = pers.tile([1, TQ], bf16, tag="mrow")
            nc.vector.tensor_copy(mrow[:], mrow_f[:])
            bq_t = pers.tile([P, NJ], f32, tag="bq_t")
            nc.sync.dma_start(bq_t[:],
                              fxt[0:1, :].rearrange("a (j p) -> p (a j)", p=P))
            bk_t = pers.tile([P, NJ], f32, tag="bk_t")
            nc.sync.dma_start(bk_t[:],
                              fxt[1:2, :].rearrange("a (j p) -> p (a j)", p=P))
            mask_bc = pers.tile([P, TQ], bf16, tag="mask_bc")
            nc.gpsimd.partition_broadcast(mask_bc[:], mrow[:])
            # xT[j]: [128 (c-chunk j), T] bf16 via DMA xbar transpose from
            # DRAM — issued first so the SP queue isn't blocked by the small
            # loads below (PE's first qk chains wait on these)
            xt = []
            for j in range(NJ):
                t_ = bigp.tile([P, T], bf16, tag=f"xt{j}")
                nc.sync.dma_start_transpose(t_[:], xbf[:, j * P:(j + 1) * P])
                xt.append(t_)

            # prefetch Q/K weight blocks for chunks 0,1 ahead of the
            # small loads: the first PE chains wait on these DMAs
            pre_w = {}
            for _pj in (0, 1):
                _wqa = wslp.tile([P, C], bf16, tag="wq_all", name=f"pw_q{_pj}")
                nc.sync.dma_start(
                    _wqa[:].rearrange("p (i c) -> p i c", c=P),
                    wq[:, _pj * P:(_pj + 1) * P].rearrange(
                        "(i p) c -> p i c", p=P))
                _wka = wslp.tile([P, C], bf16, tag="wk_all", name=f"pw_k{_pj}")
                nc.sync.dma_start(
                    _wka[:].rearrange("p (i c) -> p i c", c=P),
                    wk[:, _pj * P:(_pj + 1) * P].rearrange(
                        "(i p) c -> p i c", p=P))
                pre_w[_pj] = (_wqa, _wka)
            wv_sb0 = []
            for _i in range(NJ):
                _w = wbigp.tile([P, C], bf16, tag=f"wbig{_i}")
                nc.sync.dma_start(_w[:], wv[_i * P:(_i + 1) * P, :])
                wv_sb0.append(_w)

            # ---- phase A: small loads, broadcasts ----
            bvrow = pers.tile([1, C], f32, tag="bvrow")
            nc.sync.dma_start(bvrow[:], fxt[2:3, :])
            bprow = pers.tile([1, C], f32, tag="bprow")
            nc.sync.dma_start(bprow[:], fxt[3:4, :])

            eps_t = pers.tile([P, 1], f32, tag="eps_t")
            nc.gpsimd.memset(eps_t[:], LN_EPS)
            bv_bc = pers.tile([P, C], f32, tag="bv_bc")
            nc.gpsimd.partition_broadcast(bv_bc[:], bvrow[:])
            bp_bc = pers.tile([P, C], f32, tag="bp_bc")
            nc.gpsimd.partition_broadcast(bp_bc[:], bprow[:])
            if affine:
                lngrow = pers.tile([1, C], f32, tag="lngrow")
                nc.sync.dma_start(lngrow[:], fxt[4:5, :])
                lnbrow = pers.tile([1, C], f32, tag="lnbrow")
                nc.sync.dma_start(lnbrow[:], fxt[5:6, :])
                lng_bc = pers.tile([P, C], f32, tag="lng_bc")
                nc.gpsimd.partition_broadcast(lng_bc[:], lngrow[:])
                lnb_bc = pers.tile([P, C], f32, tag="lnb_bc")
                nc.gpsimd.partition_broadcast(lnb_bc[:], lnbrow[:])

            # ---- persistent attention operands ----
            qt = [pers.tile([P, TQ], bf16, tag=f"qt{j}", name=f"qt{j}")
                  for j in range(NJ)]
            kt = [pers.tile([P, T], bf16, tag=f"kt{j}", name=f"kt{j}")
                  for j in range(NJ)]
            vaug = [pers.tile([P, H * VSLOT], bf16, tag=f"va{t}", name=f"va{t}")
                    for t in range(NTK)]
            yt = [pers.tile([P, TQ], bf16, tag=f"yt{j}", name=f"yt{j}")
                  for j in range(NJ)]

            def emit(rp):
                # ---- phase B2: Q^T/K^T chunk j as a list of emitters, so
                # the PE chains can be interleaved into attention tk loops
                # (PE executes in program order; a contiguous qk block would
                # starve ACT between attention chunks) ----
                def qk_pieces(j):
                    if rp == 0 and j in pre_w:
                        wq_all, wk_all = pre_w[j]
                    else:
                        wq_all = wslp.tile([P, C], bf16, tag="wq_all",
                                           name=f"{rp}_wqa{j}")
                        nc.sync.dma_start(
                            wq_all[:].rearrange("p (i c) -> p i c", c=P),
                            wq[:, j * P:(j + 1) * P].rearrange(
                                "(i p) c -> p i c", p=P))
                        wk_all = wslp.tile([P, C], bf16, tag="wk_all",
                                           name=f"{rp}_wka{j}")
                        nc.sync.dma_start(
                            wk_all[:].rearrange("p (i c) -> p i c", c=P),
                            wk[:, j * P:(j + 1) * P].rearrange(
                                "(i p) c -> p i c", p=P))
                    pieces = []

                    def mk_q(blk):
                        def go():
                            psq = psp.tile([P, 512], f32, tag="mm", bufs=2,
                                           name=f"{rp}_psq{j}_{blk}")
                            for i in range(NJ):
                                nc.tensor.matmul(
                                    psq[:], wq_all[:, i * P:(i + 1) * P],
                                    xt[i][:, blk * 512:(blk + 1) * 512],
                                    start=(i == 0), stop=(i == NJ - 1))
                            # qt = (psq + bq) * mask (mask==0 rows -> q 0)
                            nc.vector.scalar_tensor_tensor(
                                qt[j][:, blk * 512:(blk + 1) * 512], psq[:],
                                bq_t[:, j:j + 1],
                                mask_bc[:, blk * 512:(blk + 1) * 512],
                                op0=ALU.add, op1=ALU.mult)
                        return go

                    def mk_k(th, blk):
                        def go():
                            psk = psp.tile([P, 512], f32, tag="mm", bufs=2,
                                           name=f"{rp}_psk{j}_{th}_{blk}")
                            for i in range(NJ):
                                nc.tensor.matmul(
                                    psk[:], wk_all[:, i * P:(i + 1) * P],
                                    xt[i][:, th * 1024 + blk * 512:
                                             th * 1024 + (blk + 1) * 512],
                                    start=(i == 0), stop=(i == NJ - 1))
                            nc.vector.tensor_scalar(
                                kt[j][:, th * 1024 + blk * 512:
                                         th * 1024 + (blk + 1) * 512], psk[:],
                                bk_t[:, j:j + 1], None, op0=ALU.add)
                        return go

                    for blk in range(2):
                        pieces.append(mk_q(blk))
                    for th in range(2):
                        for blk in range(2):
                            pieces.append(mk_k(th, blk))
                    return pieces

                def qk_produce(j):
                    for piece in qk_pieces(j):
                        piece()

                # ---- phase C: attention for (chunk j, query-half qh) ----
                # scores for both heads land in one 2-bank psum tile ->
                # single N=1024 exp ACTIVATE per tk. vaug col 0 is ones, so
                # yacc row 0 is the softmax denominator (partition 0: the
                # reciprocal+broadcast needs no partition-move DMA).
                def attn_begin(j, qh):
                    return psp.tile([65, 1024], f32, tag="yacc", bufs=1,
                                    name=f"{rp}_yacc{j}_{qh}")

                def attn_step(j, qh, yacc, tk):
                    q0 = qh * 512
                    S = psp.tile([P, 1024], f32, tag="sc", bufs=2,
                                 name=f"{rp}_S{j}_{qh}_{tk}")
                    for hh in range(2):
                        pb = hh * 64
                        nc.tensor.matmul(
                            S[:, hh * 512:(hh + 1) * 512],
                            kt[j][pb:pb + 64, tk * P:(tk + 1) * P],
                            qt[j][pb:pb + 64, q0:q0 + 512],
                            start=True, stop=True, tile_position=(pb, 0))
                    ex = evp.tile([P, 1024], bf16, tag="ex", bufs=3,
                                  name=f"{rp}_ex{j}_{qh}_{tk}")
                    nc.scalar.activation(ex[:], S[:], ACTF.Exp)
                    for hh in range(2):
                        h = 2 * j + hh
                        nc.tensor.matmul(
                            yacc[:, hh * 512:(hh + 1) * 512],
                            vaug[tk][:, h * VSLOT:h * VSLOT + 65],
                            ex[:, hh * 512:(hh + 1) * 512],
                            start=(tk == 0), stop=(tk == NTK - 1))

                def attn_finish(j, qh, yacc):
                    # copy yacc to SBUF first: the psum banks release after
                    # one DVE op (hidden under next chunk's scores+exp), and
                    # the slow normalize chain (recip -> row-64->row-0 DMA ->
                    # broadcast -> mult) runs off the critical path.
                    q0 = qh * 512
                    ycp = smp.tile([65, 1024], bf16, tag="ycp", bufs=1,
                                   name=f"{rp}_ycp{j}_{qh}")
                    nc.vector.tensor_copy(ycp[:], yacc[:])
                    for hh in range(2):
                        c0 = hh * 512
                        srr = smp.tile([65, 512], bf16, tag="srr", bufs=1,
                                       name=f"{rp}_srr{j}_{qh}_{hh}")
                        with nc.allow_low_precision(
                                reason="1/den in bf16; den~2048, tol 2e-2"):
                            nc.vector.reciprocal(srr[64:65, :],
                                                 ycp[64:65, c0:c0 + 512])
                        srb = smp.tile([1, 512], bf16, tag="srb", bufs=1,
                                       name=f"{rp}_srb{j}_{qh}_{hh}")
                        nc.sync.dma_start(srb[:], srr[64:65, :])
                        sr = smp.tile([64, 512], bf16, tag="sr", bufs=1,
                                      name=f"{rp}_sr{j}_{qh}_{hh}")
                        nc.gpsimd.partition_broadcast(sr[0:64, :], srb[:])
                        if hh == 0:
                            nc.vector.tensor_tensor(
                                yt[j][0:64, q0:q0 + 512],
                                ycp[0:64, c0:c0 + 512], sr[0:64, :],
                                op=ALU.mult)
                        else:
                            yo = smp.tile([64, 512], bf16, tag="yo", bufs=1,
                                          name=f"{rp}_yo{j}_{qh}")
                            nc.vector.tensor_tensor(
                                yo[:], ycp[0:64, c0:c0 + 512], sr[0:64, :],
                                op=ALU.mult)
                            nc.sync.dma_start(
                                yt[j][64:128, q0:q0 + 512], yo[:])

                def attn_chunk(j, qh, pieces=None):
                    yacc = attn_begin(j, qh)
                    for tk in range(NTK):
                        attn_step(j, qh, yacc, tk)
                        if pieces and tk in (2, 7, 12):
                            pieces.pop(0)()
                    attn_finish(j, qh, yacc)

                if phase_lim >= 2:
                    qk_produce(0)
                    qk_produce(1)

                # ---- phase B1: V = x @ Wv + bv into vaug (ones col 0),
                # interleaved with attn(0,0) steps ----
                if phase_lim >= 1:
                    if rp == 0:
                        wv_sb = wv_sb0
                    else:
                        wv_sb = []
                        for i in range(NJ):
                            w_ = wbigp.tile([P, C], bf16, tag=f"wbig{i}")
                            nc.sync.dma_start(w_[:],
                                              wv[i * P:(i + 1) * P, :])
                            wv_sb.append(w_)
                    for tk in range(NTK):
                        ones_ap = vaug[tk][:].rearrange("p (h e) -> p h e",
                                                        e=VSLOT)
                        nc.gpsimd.memset(ones_ap[:, :, 64:65], 1.0)
                    yacc00 = attn_begin(0, 0) if phase_lim >= 3 else None
                    for tk in range(NTK):
                        for d2 in range(2):
                            psv = psp.tile([P, 512], f32, tag="mm", bufs=2,
                                           name=f"{rp}_psv{tk}_{d2}")
                            for i in range(NJ):
                                nc.tensor.matmul(
                                    psv[:], xt[i][:, tk * P:(tk + 1) * P],
                                    wv_sb[i][:, d2 * 512:(d2 + 1) * 512],
                                    start=(i == 0), stop=(i == NJ - 1))
                            dst = vaug[tk][:].rearrange("p (h e) -> p h e",
                                                        e=VSLOT)
                            nc.vector.tensor_tensor(
                                dst[:, 8 * d2:8 * d2 + 8, 0:64],
                                psv[:].rearrange("p (h d) -> p h d", d=D),
                                bv_bc[:, d2 * 512:(d2 + 1) * 512].rearrange(
                                    "p (h d) -> p h d", d=D),
                                op=ALU.add)
                        if yacc00 is not None:
                            attn_step(0, 0, yacc00, tk)
                    if yacc00 is not None:
                        attn_finish(0, 0, yacc00)

                if phase_lim >= 3:
                    pending = qk_pieces(2)
                    attn_chunk(0, 1, pending)
                    for j in range(1, NJ):
                        if j + 2 < NJ:
                            pending.extend(qk_pieces(j + 2))
                        attn_chunk(j, 0, pending)
                        attn_chunk(j, 1, pending)

                # ---- phase D: out proj + residual + LayerNorm ----
                if phase_lim >= 4:
                    wp_sb = []
                    for i in range(NJ):
                        w_ = wbigp.tile([P, C], bf16, tag=f"wbig{i}")
                        nc.sync.dma_start(w_[:], wp[i * P:(i + 1) * P, :])
                        wp_sb.append(w_)
                    for i in range(T // P // 2):  # 8 row-tiles of TQ rows
                        xr = bigp.tile([P, C], f32, tag=f"xr{i % 2}", bufs=1,
                                       name=f"{rp}_xr{i}")
                        nc.sync.dma_start(xr[:], xres[i * P:(i + 1) * P, :])
                        hres = evp.tile([P, C], f32, tag="hres", bufs=2)
                        for half in range(2):
                            pso = psp.tile([P, 512], f32, tag="mm", bufs=2,
                                           name=f"{rp}_pso{i}_{half}")
                            for j in range(NJ):
                                nc.tensor.matmul(
                                    pso[:],
                                    yt[j][:, i * P:(i + 1) * P],
                                    wp_sb[j][:, half * 512:(half + 1) * 512],
                                    start=(j == 0), stop=(j == NJ - 1))
                            nc.vector.tensor_tensor(
                                hres[:, half * 512:(half + 1) * 512], pso[:],
                                bp_bc[:, half * 512:(half + 1) * 512],
                                op=ALU.add)
                        nc.gpsimd.tensor_tensor(hres[:], hres[:], xr[:],
                                                op=ALU.add)
                        stat = smp.tile([P, 8], f32, tag="stat")
                        sq = evp.tile([P, C], bf16, tag="sq", bufs=2)
                        nc.scalar.activation(sq[:], hres[:], ACTF.Copy,
                                             accum_out=stat[:, 0:1])
                        nc.scalar.activation(sq[:], hres[:], ACTF.Square,
                                             accum_out=stat[:, 1:2])
                        # mu, m2, var
                        nc.vector.tensor_scalar(stat[:, 2:3], stat[:, 0:1],
                                                1.0 / C, None, op0=ALU.mult)
                        nc.vector.tensor_scalar(stat[:, 3:4], stat[:, 1:2],
                                                1.0 / C, None, op0=ALU.mult)
                        nc.vector.tensor_tensor(stat[:, 4:5], stat[:, 2:3],
                                                stat[:, 2:3], op=ALU.mult)
                        nc.vector.tensor_tensor(stat[:, 5:6], stat[:, 3:4],
                                                stat[:, 4:5],
                                                op=ALU.subtract)
                        nc.scalar.activation(stat[:, 6:7], stat[:, 5:6],
                                             ACTF.Sqrt, bias=eps_t[:])
                        nc.vector.reciprocal(stat[:, 7:8], stat[:, 6:7])
                        nc.vector.tensor_scalar(hres[:], hres[:],
                                                stat[:, 2:3], stat[:, 7:8],
                                                op0=ALU.subtract,
                                                op1=ALU.mult)
                        if affine:
                            nc.vector.tensor_tensor(hres[:], hres[:],
                                                    lng_bc[:], op=ALU.mult)
                            nc.vector.tensor_tensor(hres[:], hres[:],
                                                    lnb_bc[:], op=ALU.add)
                        nc.sync.dma_start(outd[i * P:(i + 1) * P, :], hres[:])

            for _rep in range(n_reps):
                emit(_rep)

    nc.compile()
    return nc


_CACHE = {}


def _get_nc(affine: bool):
    if affine not in _CACHE:
        _CACHE[affine] = build(affine)
    return _CACHE[affine]


def _make_in_maps(x, Wq, bq, Wk, bk, Wv, bv, Wp, bp, ln_g, ln_b, mask,
                  affine: bool):
    bf = mybir.dt.np(bf16)
    sc = np.float32(1.0 / np.sqrt(D))
    w4_h = np.concatenate([
        np.asarray(Wq, np.float32) * sc, np.asarray(Wk, np.float32),
        np.asarray(Wv, np.float32), np.asarray(Wp, np.float32)],
        axis=0).astype(bf)
    x = np.asarray(x, np.float32)
    mask = np.asarray(mask)
    extra = np.stack([
        np.asarray(bq, np.float32) * sc, np.asarray(bk, np.float32),
        np.asarray(bv, np.float32), np.asarray(bp, np.float32),
        np.asarray(ln_g, np.float32), np.asarray(ln_b, np.float32),
        np.zeros(C, np.float32)], axis=0)
    in_maps = []
    for c in range(N_CORES):
        b, half = c // 2, c % 2
        xb = x[b]
        fxt_h = extra.copy()
        fxt_h[6, :] = 0.0
        fxt_h[6, :TQ] = (mask[b, half * TQ:(half + 1) * TQ] != 0)
        m = {
            "xbf": np.roll(xb, -half * TQ, axis=0).astype(bf),
            "w4": w4_h,
            "fx0": np.ascontiguousarray(xb[half * TQ:(half + 1) * TQ]),
            "fxt": fxt_h,
        }
        in_maps.append(m)
    return in_maps


def run(inputs: dict, trace: bool = False):
    ln_g = np.asarray(inputs["ln_g"], np.float32)
    ln_b = np.asarray(inputs["ln_b"], np.float32)
    affine = not (np.all(ln_g == 1.0) and np.all(ln_b == 0.0))
    nc = _get_nc(affine)
    in_maps = _make_in_maps(**inputs, affine=affine)
    res = None
    for attempt in range(3):
        try:
            res = run_bass_kernel_spmd(nc, in_maps, list(range(N_CORES)),
                                       trace=trace)
            break
        except Exception:
            if attempt == 2:
                raise
            import time as _time
            _time.sleep(2.0)
    out = np.empty((B, T, C), np.float32)
    for c in range(N_CORES):
        b, half = c // 2, c % 2
        out[b, half * TQ:(half + 1) * TQ] = res.results[c]["out"]
    return out, res


def kernel(**inputs) -> np.ndarray:
    out, _ = run(inputs, trace=False)
    return out


# revision 35
# speedup vs baseline: 1.6661x; 1.6661x over previous
"""MHA layer (QKV proj + masked softmax attention + out proj + residual + LayerNorm)
on 8 NeuronCores. Sharding: batch(4) x query-half(2). No collectives: each core
computes K/V for its full batch, Q only for its half of T.

Self-contained: hardcodes shapes from the problem spec.
"""

import numpy as np

import concourse.bass as bass
import concourse.bacc as bacc
import concourse.tile as tile
import concourse.mybir as mybir
from concourse.bass_utils import run_bass_kernel_spmd

B, T, C, H, D = 4, 2048, 1024, 16, 64
TQ = T // 2          # query rows per core
N_CORES = 8
P = 128
NJ = C // P          # 8 c-chunks
NTK = T // P         # 16 key chunks
LN_EPS = 1e-5
VSLOT = 66           # V_aug per-head slot: 64 V cols + 1 ones + 1 pad

f32 = mybir.dt.float32
bf16 = mybir.dt.bfloat16
AX = mybir.AxisListType
ALU = mybir.AluOpType
ACTF = mybir.ActivationFunctionType


def build(affine: bool):
    import os as _os0
    phase_lim = int(_os0.environ.get("K_PHASE", "4"))
    n_reps = int(_os0.environ.get("K_REPS", "1"))
    nc = bacc.Bacc("TRN2", target_bir_lowering=False, debug=False,
                   num_devices=N_CORES)

    xbf = nc.dram_tensor("xbf", [T, C], bf16, kind="ExternalInput")
    w4 = nc.dram_tensor("w4", [4 * C, C], bf16, kind="ExternalInput")
    # fx0: xres rows; fxt rows: 0 bq; 1 bk; 2 bv; 3 bp; 4 lng; 5 lnb; 6 mask
    fx0 = nc.dram_tensor("fx0", [TQ, C], f32, kind="ExternalInput")
    fxt = nc.dram_tensor("fxt", [7, C], f32, kind="ExternalInput")
    wq = w4[0 * C:1 * C, :]
    wk = w4[1 * C:2 * C, :]
    wv = w4[2 * C:3 * C, :]
    wp = w4[3 * C:4 * C, :]
    xres = fx0[0:TQ, :]
    outd = nc.dram_tensor("out", [TQ, C], f32, kind="ExternalOutput")

    with tile.TileContext(nc) as tc:
        with (
            tc.tile_pool(name="pers", bufs=1) as pers,
            tc.tile_pool(name="big", bufs=1) as bigp,
            tc.tile_pool(name="wbig", bufs=1) as wbigp,
            tc.tile_pool(name="wsl", bufs=2) as wslp,
            tc.tile_pool(name="ev", bufs=2) as evp,
            tc.tile_pool(name="sm", bufs=2) as smp,
            tc.tile_pool(name="psum", bufs=1, space=bass.MemorySpace.PSUM) as psp,
        ):
            mrow_f = evp.tile([1, TQ], f32, tag="hres", bufs=3, name="mrow_f")
            nc.sync.dma_start(mrow_f[:], fxt[6:7, :])
            mrow = pers.tile([1, TQ], bf16, tag="mrow")
            nc.vector.tensor_copy(mrow[:], mrow_f[:])
            bq_t = pers.tile([P, NJ], f32, tag="bq_t")
            nc.sync.dma_start(bq_t[:],
                              fxt[0:1, :].rearrange("a (j p) -> p (a j)", p=P))
            bk_t = pers.tile([P, NJ], f32, tag="bk_t")
            nc.sync.dma_start(bk_t[:],
                              fxt[1:2, :].rearrange("a (j p) -> p (a j)", p=P))
            mask_bc = pers.tile([P, TQ], bf16, tag="mask_bc")
            nc.gpsimd.partition_broadcast(mask_bc[:], mrow[:])
            # xT[j]: [128 (c-chunk j), T] bf16 via DMA xbar transpose from
            # DRAM — issued first so the SP queue isn't blocked by the small
            # loads below (PE's first qk chains wait on these)
            xt = []
            for j in range(NJ):
                t_ = bigp.tile([P, T], bf16, tag=f"xt{j}")
                nc.sync.dma_start_transpose(t_[:], xbf[:, j * P:(j + 1) * P])
                xt.append(t_)

            # prefetch Q/K weight blocks for chunks 0,1 ahead of the
            # small loads: the first PE chains wait on these DMAs
            pre_w = {}
            for _pj in (0, 1):
                _wqa = wslp.tile([P, C], bf16, tag="wq_all", name=f"pw_q{_pj}")
                nc.sync.dma_start(
                    _wqa[:].rearrange("p (i c) -> p i c", c=P),
                    wq[:, _pj * P:(_pj + 1) * P].rearrange(
                        "(i p) c -> p i c", p=P))
                _wka = wslp.tile([P, C], bf16, tag="wk_all", name=f"pw_k{_pj}")
                nc.sync.dma_start(
                    _wka[:].rearrange("p (i c) -> p i c", c=P),
                    wk[:, _pj * P:(_pj + 1) * P].rearrange(
                        "(i p) c -> p i c", p=P))
                pre_w[_pj] = (_wqa, _wka)
            wv_sb0 = []
            for _i in range(NJ):
                _w = wbigp.tile([P, C], bf16, tag=f"wbig{_i}")
                nc.sync.dma_start(_w[:], wv[_i * P:(_i + 1) * P, :])
                wv_sb0.append(_w)

            # ---- phase A: small loads, broadcasts ----
            bvrow = evp.tile([1, C], f32, tag="sq", bufs=2, name="bvrow")
            nc.sync.dma_start(bvrow[:], fxt[2:3, :])
            bprow = evp.tile([1, C], f32, tag="sq", bufs=2, name="bprow")
            nc.sync.dma_start(bprow[:], fxt[3:4, :])

            eps_t = pers.tile([P, 1], f32, tag="eps_t")
            nc.gpsimd.memset(eps_t[:], LN_EPS)
            bv_bc = pers.tile([P, C], f32, tag="bv_bc")
            nc.gpsimd.partition_broadcast(bv_bc[:], bvrow[:])
            bp_bc = pers.tile([P, C], f32, tag="bp_bc")
            nc.gpsimd.partition_broadcast(bp_bc[:], bprow[:])
            if affine:
                lngrow = pers.tile([1, C], f32, tag="lngrow")
                nc.sync.dma_start(lngrow[:], fxt[4:5, :])
                lnbrow = pers.tile([1, C], f32, tag="lnbrow")
                nc.sync.dma_start(lnbrow[:], fxt[5:6, :])
                lng_bc = pers.tile([P, C], f32, tag="lng_bc")
                nc.gpsimd.partition_broadcast(lng_bc[:], lngrow[:])
                lnb_bc = pers.tile([P, C], f32, tag="lnb_bc")
                nc.gpsimd.partition_broadcast(lnb_bc[:], lnbrow[:])

            # ---- persistent attention operands ----
            qt = [pers.tile([P, TQ], bf16, tag=f"qt{j}", name=f"qt{j}")
                  for j in range(NJ)]
            kt = [pers.tile([P, T], bf16, tag=f"kt{j}", name=f"kt{j}")
                  for j in range(NJ)]
            vaug = [pers.tile([P, H * VSLOT], bf16, tag=f"va{t}", name=f"va{t}")
                    for t in range(NTK)]
            yt = [pers.tile([P, TQ], bf16, tag=f"yt{j}", name=f"yt{j}")
                  for j in range(NJ)]

            def emit(rp):
                # ---- phase B2: Q^T/K^T chunk j as a list of emitters, so
                # the PE chains can be interleaved into attention tk loops
                # (PE executes in program order; a contiguous qk block would
                # starve ACT between attention chunks) ----
                def qk_pieces(j):
                    if rp == 0 and j in pre_w:
                        wq_all, wk_all = pre_w[j]
                    else:
                        wq_all = wslp.tile([P, C], bf16, tag="wq_all",
                                           name=f"{rp}_wqa{j}")
                        nc.sync.dma_start(
                            wq_all[:].rearrange("p (i c) -> p i c", c=P),
                            wq[:, j * P:(j + 1) * P].rearrange(
                                "(i p) c -> p i c", p=P))
                        wk_all = wslp.tile([P, C], bf16, tag="wk_all",
                                           name=f"{rp}_wka{j}")
                        nc.sync.dma_start(
                            wk_all[:].rearrange("p (i c) -> p i c", c=P),
                            wk[:, j * P:(j + 1) * P].rearrange(
                                "(i p) c -> p i c", p=P))
                    pieces = []

                    def mk_q(blk):
                        def go():
                            psq = psp.tile([P, 512], f32, tag="mm", bufs=2,
                                           name=f"{rp}_psq{j}_{blk}")
                            for i in range(NJ):
                                nc.tensor.matmul(
                                    psq[:], wq_all[:, i * P:(i + 1) * P],
                                    xt[i][:, blk * 512:(blk + 1) * 512],
                                    start=(i == 0), stop=(i == NJ - 1))
                            # qt = (psq + bq) * mask (mask==0 rows -> q 0)
                            nc.vector.scalar_tensor_tensor(
                                qt[j][:, blk * 512:(blk + 1) * 512], psq[:],
                                bq_t[:, j:j + 1],
                                mask_bc[:, blk * 512:(blk + 1) * 512],
                                op0=ALU.add, op1=ALU.mult)
                        return go

                    def mk_k(th, blk):
                        def go():
                            psk = psp.tile([P, 512], f32, tag="mm", bufs=2,
                                           name=f"{rp}_psk{j}_{th}_{blk}")
                            for i in range(NJ):
                                nc.tensor.matmul(
                                    psk[:], wk_all[:, i * P:(i + 1) * P],
                                    xt[i][:, th * 1024 + blk * 512:
                                             th * 1024 + (blk + 1) * 512],
                                    start=(i == 0), stop=(i == NJ - 1))
                            nc.vector.tensor_scalar(
                                kt[j][:, th * 1024 + blk * 512:
                                         th * 1024 + (blk + 1) * 512], psk[:],
                                bk_t[:, j:j + 1], None, op0=ALU.add)
                        return go

                    for blk in range(2):
                        pieces.append(mk_q(blk))
                    for th in range(2):
                        for blk in range(2):
                            pieces.append(mk_k(th, blk))
                    return pieces

                def qk_produce(j):
                    for piece in qk_pieces(j):
                        piece()

                # ---- phase C: attention for (chunk j, query-half qh) ----
                # scores for both heads land in one 2-bank psum tile ->
                # single N=1024 exp ACTIVATE per tk. vaug col 0 is ones, so
                # yacc row 0 is the softmax denominator (partition 0: the
                # reciprocal+broadcast needs no partition-move DMA).
                def attn_begin(j, qh):
                    return psp.tile([65, 1024], f32, tag="yacc", bufs=1,
                                    name=f"{rp}_yacc{j}_{qh}")

                def attn_step(j, qh, yacc, tk):
                    q0 = qh * 512
                    S = psp.tile([P, 1024], f32, tag="sc", bufs=2,
                                 name=f"{rp}_S{j}_{qh}_{tk}")
                    for hh in range(2):
                        pb = hh * 64
                        nc.tensor.matmul(
                            S[:, hh * 512:(hh + 1) * 512],
                            kt[j][pb:pb + 64, tk * P:(tk + 1) * P],
                            qt[j][pb:pb + 64, q0:q0 + 512],
                            start=True, stop=True, tile_position=(pb, 0))
                    ex = evp.tile([P, 1024], bf16, tag="ex", bufs=3,
                                  name=f"{rp}_ex{j}_{qh}_{tk}")
                    nc.scalar.activation(ex[:], S[:], ACTF.Exp)
                    for hh in range(2):
                        h = 2 * j + hh
                        nc.tensor.matmul(
                            yacc[:, hh * 512:(hh + 1) * 512],
                            vaug[tk][:, h * VSLOT:h * VSLOT + 65],
                            ex[:, hh * 512:(hh + 1) * 512],
                            start=(tk == 0), stop=(tk == NTK - 1))

                def attn_finish(j, qh, yacc):
                    # copy yacc to SBUF first: the psum banks release after
                    # one DVE op (hidden under next chunk's scores+exp), and
                    # the slow normalize chain (recip -> row-64->row-0 DMA ->
                    # broadcast -> mult) runs off the critical path.
                    q0 = qh * 512
                    ycp = smp.tile([65, 1024], bf16, tag="ycp", bufs=1,
                                   name=f"{rp}_ycp{j}_{qh}")
                    nc.vector.tensor_copy(ycp[:], yacc[:])
                    for hh in (1, 0):
                        c0 = hh * 512
                        srr = smp.tile([65, 512], bf16, tag="srr", bufs=1,
                                       name=f"{rp}_srr{j}_{qh}_{hh}")
                        with nc.allow_low_precision(
                                reason="1/den in bf16; den~2048, tol 2e-2"):
                            nc.vector.reciprocal(srr[64:65, :],
                                                 ycp[64:65, c0:c0 + 512])
                        srb = smp.tile([1, 512], bf16, tag="srb", bufs=1,
                                       name=f"{rp}_srb{j}_{qh}_{hh}")
                        nc.sync.dma_start(srb[:], srr[64:65, :])
                        sr = smp.tile([64, 512], bf16, tag="sr", bufs=1,
                                      name=f"{rp}_sr{j}_{qh}_{hh}")
                        nc.gpsimd.partition_broadcast(sr[0:64, :], srb[:])
                        if hh == 0:
                            nc.vector.tensor_tensor(
                                yt[j][0:64, q0:q0 + 512],
                                ycp[0:64, c0:c0 + 512], sr[0:64, :],
                                op=ALU.mult)
                        else:
                            yo = smp.tile([64, 512], bf16, tag="yo", bufs=1,
                                          name=f"{rp}_yo{j}_{qh}")
                            nc.vector.tensor_tensor(
                                yo[:], ycp[0:64, c0:c0 + 512], sr[0:64, :],
                                op=ALU.mult)
                            nc.sync.dma_start(
                                yt[j][64:128, q0:q0 + 512], yo[:])

                def attn_chunk(j, qh, pieces=None):
                    yacc = attn_begin(j, qh)
                    for tk in range(NTK):
                        attn_step(j, qh, yacc, tk)
                        if pieces and tk in (4, 9, 14):
                            pieces.pop(0)()
                    attn_finish(j, qh, yacc)

                if phase_lim >= 2:
                    qk_produce(0)
                    qk_produce(1)

                # ---- phase B1: V = x @ Wv + bv into vaug (ones col 0),
                # interleaved with attn(0,0) steps ----
                if phase_lim >= 1:
                    if rp == 0:
                        wv_sb = wv_sb0
                    else:
                        wv_sb = []
                        for i in range(NJ):
                            w_ = wbigp.tile([P, C], bf16, tag=f"wbig{i}")
                            nc.sync.dma_start(w_[:],
                                              wv[i * P:(i + 1) * P, :])
                            wv_sb.append(w_)
                    for tk in range(NTK):
                        ones_ap = vaug[tk][:].rearrange("p (h e) -> p h e",
                                                        e=VSLOT)
                        nc.gpsimd.memset(ones_ap[:, :, 64:65], 1.0)
                    # two passes over tk: d2 half 0 with attn(0,0) steps,
                    # then d2 half 1 with attn(0,1) steps — attn(0,0) only
                    # reads heads 0/1 which the d2=0 half provides, so both
                    # attention chunks of j=0 hide inside the V phase.
                    def v_half(tk, d2):
                        psv = psp.tile([P, 512], f32, tag="mm", bufs=2,
                                       name=f"{rp}_psv{tk}_{d2}")
                        for i in range(NJ):
                            nc.tensor.matmul(
                                psv[:], xt[i][:, tk * P:(tk + 1) * P],
                                wv_sb[i][:, d2 * 512:(d2 + 1) * 512],
                                start=(i == 0), stop=(i == NJ - 1))
                        dst = vaug[tk][:].rearrange("p (h e) -> p h e",
                                                    e=VSLOT)
                        nc.vector.tensor_tensor(
                            dst[:, 8 * d2:8 * d2 + 8, 0:64],
                            psv[:].rearrange("p (h d) -> p h d", d=D),
                            bv_bc[:, d2 * 512:(d2 + 1) * 512].rearrange(
                                "p (h d) -> p h d", d=D),
                            op=ALU.add)

                    yacc00 = attn_begin(0, 0) if phase_lim >= 3 else None
                    for tk in range(NTK):
                        v_half(tk, 0)
                        if yacc00 is not None:
                            attn_step(0, 0, yacc00, tk)
                    if yacc00 is not None:
                        attn_finish(0, 0, yacc00)
                    yacc01 = attn_begin(0, 1) if phase_lim >= 3 else None
                    for tk in range(NTK):
                        v_half(tk, 1)
                        if yacc01 is not None:
                            attn_step(0, 1, yacc01, tk)
                    if yacc01 is not None:
                        attn_finish(0, 1, yacc01)

                if phase_lim >= 3:
                    pending = qk_pieces(2)
                    for j in range(1, NJ):
                        if j + 2 < NJ:
                            pending.extend(qk_pieces(j + 2))
                        attn_chunk(j, 0, pending)
                        attn_chunk(j, 1, pending)

                # ---- phase D: out proj + residual + LayerNorm ----
                if phase_lim >= 4:
                    wp_sb = []
                    for i in range(NJ):
                        w_ = wbigp.tile([P, C], bf16, tag=f"wbig{i}")
                        nc.sync.dma_start(w_[:], wp[i * P:(i + 1) * P, :])
                        wp_sb.append(w_)
                    for i in range(T // P // 2):  # 8 row-tiles of TQ rows
                        xr = bigp.tile([P, C], f32, tag=f"xr{i % 2}", bufs=1,
                                       name=f"{rp}_xr{i}")
                        nc.sync.dma_start(xr[:], xres[i * P:(i + 1) * P, :])
                        hres = evp.tile([P, C], f32, tag="hres", bufs=3)
                        for half in range(2):
                            pso = psp.tile([P, 512], f32, tag="mm", bufs=2,
                                           name=f"{rp}_pso{i}_{half}")
                            for j in range(NJ):
                                nc.tensor.matmul(
                                    pso[:],
                                    yt[j][:, i * P:(i + 1) * P],
                                    wp_sb[j][:, half * 512:(half + 1) * 512],
                                    start=(j == 0), stop=(j == NJ - 1))
                            nc.vector.tensor_tensor(
                                hres[:, half * 512:(half + 1) * 512], pso[:],
                                bp_bc[:, half * 512:(half + 1) * 512],
                                op=ALU.add)
                        nc.gpsimd.tensor_tensor(hres[:], hres[:], xr[:],
                                                op=ALU.add)
                        stat = smp.tile([P, 8], f32, tag="stat", bufs=4)
                        sq = evp.tile([P, C], bf16, tag="sq", bufs=2)
                        nc.scalar.activation(sq[:], hres[:], ACTF.Copy,
                                             accum_out=stat[:, 0:1])
                        nc.scalar.activation(sq[:], hres[:], ACTF.Square,
                                             accum_out=stat[:, 1:2])
                        # mu, m2, var
                        nc.vector.tensor_scalar(stat[:, 2:3], stat[:, 0:1],
                                                1.0 / C, None, op0=ALU.mult)
                        nc.vector.tensor_scalar(stat[:, 3:4], stat[:, 1:2],
                                                1.0 / C, None, op0=ALU.mult)
                        nc.vector.tensor_tensor(stat[:, 4:5], stat[:, 2:3],
                                                stat[:, 2:3], op=ALU.mult)
                        nc.vector.tensor_tensor(stat[:, 5:6], stat[:, 3:4],
                                                stat[:, 4:5],
                                                op=ALU.subtract)
                        nc.scalar.activation(stat[:, 6:7], stat[:, 5:6],
                                             ACTF.Sqrt, bias=eps_t[:])
                        nc.vector.reciprocal(stat[:, 7:8], stat[:, 6:7])
                        nc.vector.tensor_scalar(hres[:], hres[:],
                                                stat[:, 2:3], stat[:, 7:8],
                                                op0=ALU.subtract,
                                                op1=ALU.mult)
                        if affine:
                            nc.vector.tensor_tensor(hres[:], hres[:],
                                                    lng_bc[:], op=ALU.mult)
                            nc.vector.tensor_tensor(hres[:], hres[:],
                                                    lnb_bc[:], op=ALU.add)
                        nc.sync.dma_start(outd[i * P:(i + 1) * P, :], hres[:])

            for _rep in range(n_reps):
                emit(_rep)

    nc.compile()
    return nc


_CACHE = {}


def _get_nc(affine: bool):
    if affine not in _CACHE:
        _CACHE[affine] = build(affine)
    return _CACHE[affine]


def _make_in_maps(x, Wq, bq, Wk, bk, Wv, bv, Wp, bp, ln_g, ln_b, mask,
                  affine: bool):
    bf = mybir.dt.np(bf16)
    sc = np.float32(1.0 / np.sqrt(D))
    w4_h = np.concatenate([
        np.asarray(Wq, np.float32) * sc, np.asarray(Wk, np.float32),
        np.asarray(Wv, np.float32), np.asarray(Wp, np.float32)],
        axis=0).astype(bf)
    x = np.asarray(x, np.float32)
    mask = np.asarray(mask)
    extra = np.stack([
        np.asarray(bq, np.float32) * sc, np.asarray(bk, np.float32),
        np.asarray(bv, np.float32), np.asarray(bp, np.float32),
        np.asarray(ln_g, np.float32), np.asarray(ln_b, np.float32),
        np.zeros(C, np.float32)], axis=0)
    in_maps = []
    for c in range(N_CORES):
        b, half = c // 2, c % 2
        xb = x[b]
        fxt_h = extra.copy()
        fxt_h[6, :] = 0.0
        fxt_h[6, :TQ] = (mask[b, half * TQ:(half + 1) * TQ] != 0)
        m = {
            "xbf": np.roll(xb, -half * TQ, axis=0).astype(bf),
            "w4": w4_h,
            "fx0": np.ascontiguousarray(xb[half * TQ:(half + 1) * TQ]),
            "fxt": fxt_h,
        }
        in_maps.append(m)
    return in_maps


def run(inputs: dict, trace: bool = False):
    ln_g = np.asarray(inputs["ln_g"], np.float32)
    ln_b = np.asarray(inputs["ln_b"], np.float32)
    affine = not (np.all(ln_g == 1.0) and np.all(ln_b == 0.0))
    nc = _get_nc(affine)
    in_maps = _make_in_maps(**inputs, affine=affine)
    res = None
    for attempt in range(3):
        try:
            res = run_bass_kernel_spmd(nc, in_maps, list(range(N_CORES)),
                                       trace=trace)
            break
        except Exception:
            if attempt == 2:
                raise
            import time as _time
            _time.sleep(2.0)
    out = np.empty((B, T, C), np.float32)
    for c in range(N_CORES):
        b, half = c // 2, c % 2
        out[b, half * TQ:(half + 1) * TQ] = res.results[c]["out"]
    return out, res


def kernel(**inputs) -> np.ndarray:
    out, _ = run(inputs, trace=False)
    return out
